# revision 12
# baseline (speedup 1.0000x reference)
"""Trainium2 Bass kernel for the ActorCritic GNN (gnn_message_passing).

Strategy (8 NeuronCores, graph-data parallel):
  - Core c owns graphs [8c, 8c+8) == node rows [6250c, 6250(c+1)) (the batch
    vector's graph boundaries align with the 8-way node split).
  - Edges are partitioned by dst owner; all segment reductions (scatter-sum,
    GraphNorm, pooling) stay device-local.
  - Per conv layer: each core computes its local m' = (h @ W) * dinv rows,
    AllGathers the full [N, 2H] message table into DRAM, then processes its
    ~E/8 edges: bulk dma_gather of 1KB rows + one-hot scatter matmuls
    accumulating into PSUM windows of 128 dst nodes.
  - Both encoders (actor/critic) are fused: one gather serves both (rows hold
    [actor 128 | critic 128] features).
  - int16 gather indices -> the node table is processed as two halves
    (rows [0, 32768) and [32768, N)), edges bucketed by src half.
"""

import os
import numpy as np

import concourse.bacc as bacc
import concourse.bass as bass
import concourse.mybir as mybir
import concourse.tile as tile
from concourse import library_config
from concourse.bass_utils import run_bass_kernel_spmd

F32 = mybir.dt.float32
BF = mybir.dt.bfloat16
I16 = mybir.dt.int16
I32 = mybir.dt.int32
AF = mybir.ActivationFunctionType
OP = mybir.AluOpType

H = 128          # hidden width (fixed by the arch; layout assumes 128)
ND = 7
GF = 10
NA = 7
L = 3
EPS = 1e-5

FULL_CFG = dict(N=50000, E=800000, B=64, C=8, HALF=32768)


# ----------------------------------------------------------------------------
# host preprocessing
# ----------------------------------------------------------------------------

def _pack_gather_idx(e_flat):
    """Pack slot-ordered int16 indices (slot s = t*128 + p) into the
    [16, 8T] layout dma_gather reads: slot (p=16a+b, t) <- H[b, 8t+a]."""
    T = len(e_flat) // 128
    return e_flat.reshape(T, 8, 16).transpose(2, 0, 1).reshape(16, 8 * T)


def preprocess(x, edge_index, batch, global_features, params, cfg):
    N, E, B, C, HALFV = cfg["N"], cfg["E"], cfg["B"], cfg["C"], cfg["HALF"]
    NL = N // C
    GL = B // C
    NCH = (NL + 127) // 128          # windows / node chunks per core
    CWL = NL - 128 * (NCH - 1)       # width of last chunk

    src = np.asarray(edge_index[0], dtype=np.int64)
    dst = np.asarray(edge_index[1], dtype=np.int64)
    batch = np.asarray(batch, dtype=np.int64)
    x = np.asarray(x, dtype=np.float32)
    gfeat = np.asarray(global_features, dtype=np.float32)

    deg = np.bincount(dst, minlength=N).astype(np.float64) + 1.0
    dinv = (deg ** -0.5).astype(np.float32)

    # fold the GCN self-loop in as real edges: contribution dinv[d]*m'[d]
    allx = np.arange(N, dtype=np.int64)
    src = np.concatenate([src, allx])
    dst = np.concatenate([dst, allx])

    # graph boundaries; check the 8-way split is uniform and graph-aligned
    gb = np.searchsorted(batch, np.arange(B + 1))
    cs = np.array([gb[c * GL] for c in range(C)] + [N])
    assert np.all(np.diff(cs) == NL), f"non-uniform core node split {np.diff(cs)}"
    # local graph slices (must be identical across cores)
    gsl = gb[: C * GL].reshape(C, GL) - cs[:C, None]
    assert np.all(gsl == gsl[0]), "graph slices differ across cores"
    g_lo = gsl[0]
    g_hi = np.concatenate([gsl[0][1:], [NL]])
    g_cnt = (g_hi - g_lo).astype(np.float64)

    # --- edge bucketing: (core, window, src-half) ---
    core_e = dst // NL
    dstloc = dst - core_e * NL
    w_e = dstloc // 128
    off_e = (dstloc % 128).astype(np.float32)
    s_e = (src >= HALFV).astype(np.int64)

    key = (core_e * NCH + w_e) * 2 + s_e
    order = np.argsort(key, kind="stable")
    key_s = key[order]
    nbuckets = C * NCH * 2
    bstart = np.searchsorted(key_s, np.arange(nbuckets + 1))
    counts = np.diff(bstart).reshape(C, NCH, 2)

    T_star = np.ceil(counts / 128).astype(np.int64).max(axis=0)  # [NCH, 2]
    TT = int(T_star.sum())            # tiles per layer (static)
    IC = 8 * TT                       # idx columns

    src_s = src[order]
    off_s = off_e[order]

    idx_all = np.zeros((C, 16, IC), dtype=np.int16)
    dstoff = np.full((C, 128, TT), -1.0, dtype=np.float32)

    # static per-bucket offsets
    col0 = np.zeros((NCH, 2), dtype=np.int64)   # idx col starts
    tau0 = np.zeros((NCH, 2), dtype=np.int64)   # tile index starts
    acc_c, acc_t = 0, 0
    for w in range(NCH):
        for s in range(2):
            col0[w, s] = acc_c
            tau0[w, s] = acc_t
            acc_c += 8 * T_star[w, s]
            acc_t += T_star[w, s]

    for c in range(C):
        for w in range(NCH):
            for s in range(2):
                Tb = int(T_star[w, s])
                if Tb == 0:
                    continue
                b = (c * NCH + w) * 2 + s
                lo, hi = bstart[b], bstart[b + 1]
                n = hi - lo
                e_idx = np.zeros(128 * Tb, dtype=np.int16)
                e_off = np.full(128 * Tb, -1.0, dtype=np.float32)
                vals = src_s[lo:hi] - (HALFV if s == 1 else 0)
                e_idx[:n] = vals.astype(np.int16)
                e_off[:n] = off_s[lo:hi]
                cc = int(col0[w, s])
                for t0 in range(0, Tb, 8):
                    ts = min(8, Tb - t0)
                    idx_all[c, :, cc + 8 * t0 : cc + 8 * (t0 + ts)] = (
                        _pack_gather_idx(e_idx[128 * t0 : 128 * (t0 + ts)])
                    )
                tt = int(tau0[w, s])
                dstoff[c, :, tt : tt + Tb] = e_off.reshape(Tb, 128).T

    idx_all_full = np.tile(idx_all, (1, 8, 1))  # [C, 128, IC]

    # --- per-core dense inputs ---
    xT = np.stack([x[cs[c] : cs[c + 1]].T for c in range(C)])           # [C,7,NL]
    gfT = np.stack([gfeat[c * GL : (c + 1) * GL].T for c in range(C)])  # [C,GF,GL]
    dinv_nd = np.zeros((C, 128, NCH), dtype=np.float32)
    for c in range(C):
        dv = dinv[cs[c] : cs[c + 1]]
        pad = np.zeros(NCH * 128, dtype=np.float32)
        pad[:NL] = dv
        dinv_nd[c] = pad.reshape(NCH, 128).T
    cntinv = np.broadcast_to(
        (1.0 / g_cnt).astype(np.float32)[None, :], (128, GL)
    ).copy()

    # --- parameters (replicated) ---
    def A(v):
        return np.ascontiguousarray(np.asarray(v, dtype=np.float32))

    pa, pc = params["actor_enc"], params["critic_enc"]
    inW = np.concatenate([A(pa["in_W"]), A(pc["in_W"])], axis=1)           # [7,2H]
    inb = np.stack([A(pa["in_b"]), A(pc["in_b"])], axis=1)                 # [H,2]
    convW = np.concatenate(
        [A(p["conv_W"][l]) for l in range(L) for p in (pa, pc)], axis=1
    )                                                                       # [H, L*2*H]
    convB = np.stack(
        [A(p["conv_b"][l]) for l in range(L) for p in (pa, pc)], axis=1
    )                                                                       # [H, L*2]
    normP = np.stack(
        [
            A(p[k][l])
            for l in range(L)
            for p in (pa, pc)
            for k in ("norm_a", "norm_w", "norm_b")
        ],
        axis=1,
    )                                                                       # [H, L*2*3]

    ha, hc = params["actor_head"], params["critic_head"]
    w1 = np.concatenate(
        [A(h_["W1"])[k * H : (k + 1) * H] for h_ in (ha, hc) for k in (0, 1)],
        axis=1,
    )                                                                       # [H, 4H]
    w1g = np.concatenate([A(ha["W1"])[2 * H :], A(hc["W1"])[2 * H :]], axis=1)  # [GF,2H]
    b1 = np.stack([A(ha["b1"]), A(hc["b1"])], axis=1)                      # [H,2]
    w2 = np.concatenate([A(ha["W2"]), A(hc["W2"])], axis=1)                # [H,128]
    b2 = np.stack([A(ha["b2"]), A(hc["b2"])], axis=1)                      # [64,2]
    w3 = np.concatenate([A(ha["W3"]), A(hc["W3"])], axis=1)                # [64,NA+2]
    b3a = A(ha["b3"])[:, None]                                             # [NA,1]
    b3c = A(hc["b3"])[:, None]                                             # [2,1]
    eye = np.eye(128, dtype=np.float32)

    in_maps = []
    for c in range(C):
        in_maps.append(
            {
                "xT": np.ascontiguousarray(xT[c]),
                "gfT": np.ascontiguousarray(gfT[c]),
                "idx_all": np.ascontiguousarray(idx_all_full[c]),
                "dstoff": np.ascontiguousarray(dstoff[c]),
                "dinv_nd": np.ascontiguousarray(dinv_nd[c]),
                "cntinv": cntinv,
                "inW": inW, "inb": inb, "convW": convW, "convB": convB,
                "normP": normP, "w1": w1, "w1g": w1g, "b1": b1,
                "w2": w2, "b2": b2, "w3": w3, "b3a": b3a, "b3c": b3c, "eye": eye,
            }
        )

    meta = dict(
        NL=NL, GL=GL, NCH=NCH, CWL=CWL, TT=TT, IC=IC,
        T_star=T_star, col0=col0, tau0=tau0,
        g_lo=g_lo.tolist(), g_hi=g_hi.tolist(),
        N=N, C=C, HALF=HALFV,
    )
    return in_maps, meta


# ----------------------------------------------------------------------------
# device kernel
# ----------------------------------------------------------------------------

def build_kernel(meta):
    NL, GL, NCH, CWL = meta["NL"], meta["GL"], meta["NCH"], meta["CWL"]
    TT, IC = meta["TT"], meta["IC"]
    T_star, col0, tau0 = meta["T_star"], meta["col0"], meta["tau0"]
    g_lo, g_hi = meta["g_lo"], meta["g_hi"]
    N, C, HALFV = meta["N"], meta["C"], meta["HALF"]
    H2 = 2 * H
    NHEAD = NA + NA + 1 + 1 + 2 * H2   # logits, probs, value, term, a_emb, c_emb

    nc = bacc.Bacc("TRN2", target_bir_lowering=False, debug=True)

    def P(name, shape, dtype=F32):
        return nc.declare_dram_parameter(name, shape, dtype, isOutput=False)

    xT_d = P("xT", [ND, NL])
    gfT_d = P("gfT", [GF, GL])
    idx_d = P("idx_all", [128, IC], I16)
    dstoff_d = P("dstoff", [128, TT])
    dinv_d = P("dinv_nd", [128, NCH])
    cntinv_d = P("cntinv", [128, GL])
    inW_d = P("inW", [ND, H2])
    inb_d = P("inb", [H, 2])
    convW_d = P("convW", [H, L * 2 * H])
    convB_d = P("convB", [H, L * 2])
    normP_d = P("normP", [H, L * 6])
    w1_d = P("w1", [H, 4 * H])
    w1g_d = P("w1g", [GF, H2])
    b1_d = P("b1", [H, 2])
    w2_d = P("w2", [H, 128])
    b2_d = P("b2", [64, 2])
    w3_d = P("w3", [64, NA + 2])
    b3a_d = P("b3a", [NA, 1])
    b3c_d = P("b3c", [2, 1])
    eye_d = P("eye", [128, 128])

    nodes_out = nc.declare_dram_parameter("nodes_out", [NL, H2], F32, isOutput=True)
    heads_out = nc.declare_dram_parameter("heads_out", [GL, NHEAD], F32, isOutput=True)

    shard = [nc.dram_tensor(f"shard{l}", [NL, H2], BF) for l in range(L)]
    mtab = [
        nc.dram_tensor(f"mtab{l}", [N, H2], BF, addr_space="Shared")
        for l in range(L)
    ]

    rg = [list(range(C))]

    with tile.TileContext(nc) as tc:
        nc.gpsimd.load_library(library_config.mlp)
        with (
            tc.tile_pool(name="const", bufs=1) as cst,
            tc.tile_pool(name="big", bufs=1) as big,
            tc.tile_pool(name="msg", bufs=2) as msgp,
            tc.tile_pool(name="oh", bufs=4) as ohp,
            tc.tile_pool(name="wrk", bufs=3) as wrk,
            tc.tile_pool(name="small", bufs=8) as smp,
            tc.tile_pool(name="psum", bufs=6, space="PSUM") as psp,
        ):
            # ---- constants to SBUF ----
            def load(dram, shape, dtype=F32):
                nm = f"c_{dram.name}"
                t = cst.tile(shape, dtype, tag=nm, name=nm)
                nc.sync.dma_start(out=t[:], in_=dram[:])
                return t

            xT = load(xT_d, [ND, NL])
            gfT = load(gfT_d, [GF, GL])
            idx_sb = load(idx_d, [128, IC], I16)
            dstoff_sb = load(dstoff_d, [128, TT])
            dinv_sb = load(dinv_d, [128, NCH])
            cntinv_sb = load(cntinv_d, [128, GL])
            inW = load(inW_d, [ND, H2])
            inb = load(inb_d, [H, 2])
            convW = load(convW_d, [H, L * 2 * H])
            convB = load(convB_d, [H, L * 2])
            normP = load(normP_d, [H, L * 6])
            w1 = load(w1_d, [H, 4 * H])
            w1g = load(w1g_d, [GF, H2])
            b1 = load(b1_d, [H, 2])
            w2 = load(w2_d, [H, 128])
            b2 = load(b2_d, [64, 2])
            w3 = load(w3_d, [64, NA + 2])
            b3a = load(b3a_d, [NA, 1])
            b3c = load(b3c_d, [2, 1])
            eye = load(eye_d, [128, 128])

            iota_i = cst.tile([128, 128], I32)
            nc.gpsimd.iota(iota_i[:], pattern=[[1, 128]], base=0, channel_multiplier=0)
            iota_b = cst.tile([128, 128], BF)
            nc.vector.tensor_copy(out=iota_b[:], in_=iota_i[:])
            eps_t = cst.tile([128, 1], F32)
            nc.vector.memset(eps_t[:], EPS)

            h = [big.tile([128, NL], F32, tag=f"h{e}", name=f"h{e}") for e in range(2)]
            aggT = [big.tile([128, NL], F32, tag=f"agg{e}", name=f"agg{e}") for e in range(2)]

            def cw_of(ch):
                return CWL if ch == NCH - 1 else 128

            # ---- input layer: h0 = relu(x @ in_W + in_b), feature-major ----
            NJ = (NL + 511) // 512
            for e in range(2):
                for j in range(NJ):
                    j0 = j * 512
                    wj = min(512, NL - j0)
                    ps = psp.tile([128, 512], F32, tag="ps")
                    nc.tensor.matmul(
                        out=ps[:, :wj],
                        lhsT=inW[:, e * H : (e + 1) * H],
                        rhs=xT[:, j0 : j0 + wj],
                        start=True, stop=True,
                    )
                    nc.scalar.activation(
                        out=h[e][:, j0 : j0 + wj], in_=ps[:, :wj],
                        func=AF.Relu, bias=inb[:, e : e + 1], scale=1.0,
                    )

            # ---- conv layers ----
            KSTAGE = int(os.environ.get("KSTAGE", "99"))
            NLAYERS = 0 if KSTAGE < 2 else (1 if KSTAGE < 6 else L)
            for l in range(NLAYERS):
                # stage B: m' = (h @ W_l) * dinv -> shard -> AllGather
                for ch in range(NCH):
                    cw = cw_of(ch)
                    msb = wrk.tile([128, H2], BF, tag="msb")
                    for e in range(2):
                        psb = psp.tile([128, 128], F32, tag="ps")
                        nc.tensor.matmul(
                            out=psb[:cw, :],
                            lhsT=h[e][:, ch * 128 : ch * 128 + cw],
                            rhs=convW[:, (l * 2 + e) * H : (l * 2 + e + 1) * H],
                            start=True, stop=True,
                        )
                        nc.scalar.activation(
                            out=msb[:cw, e * H : (e + 1) * H], in_=psb[:cw, :],
                            func=AF.Copy, scale=dinv_sb[:cw, ch : ch + 1],
                        )
                    nc.sync.dma_start(
                        out=shard[l][ch * 128 : ch * 128 + cw, :], in_=msb[:cw, :]
                    )
                if KSTAGE >= 3:
                    nc.gpsimd.collective_compute(
                        "AllGather", OP.bypass, replica_groups=rg,
                        ins=[shard[l][:]], outs=[mtab[l][:]],
                    )

                if KSTAGE < 4:
                    continue

                # stage D: edge pass per window
                for w in range(NCH):
                    cw = cw_of(w)
                    psw = psp.tile([128, H2], F32, tag="ps")
                    n_tiles = int(T_star[w, 0] + T_star[w, 1])
                    done = 0
                    for s in range(2):
                        Tb = int(T_star[w, s])
                        if Tb == 0:
                            continue
                        msg = msgp.tile([128, Tb, H2], BF, tag="msg")
                        base = (
                            mtab[l][0:HALFV, :] if s == 0 else mtab[l][HALFV:N, :]
                        )
                        c0 = int(col0[w, s])
                        for t0 in range(0, Tb, 8):
                            ts = min(8, Tb - t0)
                            nc.gpsimd.dma_gather(
                                msg[:, t0 : t0 + ts, :], base,
                                idx_sb[:, c0 + 8 * t0 : c0 + 8 * (t0 + ts)],
                                128 * ts, 128 * ts, H2,
                            )
                        t0_ = int(tau0[w, s])
                        for t in range(Tb):
                            oh = ohp.tile([128, 128], BF, tag="oh")
                            nc.vector.tensor_scalar(
                                out=oh[:], in0=iota_b[:],
                                scalar1=dstoff_sb[:, t0_ + t : t0_ + t + 1],
                                scalar2=None, op0=OP.is_equal,
                            )
                            nc.tensor.matmul(
                                out=psw[:, :], lhsT=oh[:], rhs=msg[:, t, :],
                                start=(done == 0), stop=(done == n_tiles - 1),
                            )
                            done += 1
                    # epilogue: agg = scatter * dinv  (self-loop folded in as
                    # edges; conv bias folded into the GraphNorm stats)
                    aw = wrk.tile([128, H2], F32, tag="aw")
                    nc.scalar.activation(
                        out=aw[:cw, :], in_=psw[:cw, :], func=AF.Copy,
                        scale=dinv_sb[:cw, w : w + 1],
                    )
                    for e in range(2):
                        pst = psp.tile([128, 128], F32, tag="ps")
                        nc.tensor.transpose(
                            out=pst[:, :cw],
                            in_=aw[:cw, e * H : (e + 1) * H],
                            identity=eye[:cw, :cw],
                        )
                        nc.scalar.copy(
                            out=aggT[e][:, w * 128 : w * 128 + cw], in_=pst[:, :cw]
                        )

                # stage E: GraphNorm + relu + skip (feature-major, in place)
                if KSTAGE < 5:
                    continue
                for e in range(2):
                    al = normP[:, (l * 2 + e) * 3 + 0 : (l * 2 + e) * 3 + 1]
                    wn = normP[:, (l * 2 + e) * 3 + 1 : (l * 2 + e) * 3 + 2]
                    bn = normP[:, (l * 2 + e) * 3 + 2 : (l * 2 + e) * 3 + 3]
                    bc = convB[:, l * 2 + e : l * 2 + e + 1]

                    gsum = smp.tile([128, GL], F32, tag="st")
                    for g in range(GL):
                        nc.vector.reduce_sum(
                            out=gsum[:, g : g + 1],
                            in_=aggT[e][:, g_lo[g] : g_hi[g]],
                            axis=mybir.AxisListType.X,
                        )
                    sg = smp.tile([128, GL], F32, tag="st")
                    nc.vector.tensor_tensor(
                        out=sg[:], in0=gsum[:], in1=cntinv_sb[:], op=OP.mult
                    )
                    # sg = alpha*(mean + b_conv) - b_conv
                    nc.vector.tensor_scalar(
                        out=sg[:], in0=sg[:], scalar1=bc, scalar2=al,
                        op0=OP.add, op1=OP.mult,
                    )
                    nc.vector.tensor_scalar(
                        out=sg[:], in0=sg[:], scalar1=bc, scalar2=None,
                        op0=OP.subtract,
                    )
                    for g in range(GL):
                        nc.vector.tensor_scalar(
                            out=aggT[e][:, g_lo[g] : g_hi[g]],
                            in0=aggT[e][:, g_lo[g] : g_hi[g]],
                            scalar1=sg[:, g : g + 1], scalar2=None,
                            op0=OP.subtract,
                        )
                    vsum = smp.tile([128, GL], F32, tag="st")
                    sq = wrk.tile([128, 1024], F32, tag="sq")
                    for g in range(GL):
                        nc.scalar.activation(
                            out=sq[:, : g_hi[g] - g_lo[g]],
                            in_=aggT[e][:, g_lo[g] : g_hi[g]],
                            func=AF.Square,
                            accum_out=vsum[:, g : g + 1],
                        )
                    nc.vector.tensor_tensor(
                        out=vsum[:], in0=vsum[:], in1=cntinv_sb[:], op=OP.mult
                    )
                    nc.scalar.activation(out=vsum[:], in_=vsum[:], func=AF.Sqrt, bias=eps_t[:])
                    nc.vector.reciprocal(out=vsum[:], in_=vsum[:])
                    nc.vector.tensor_scalar(
                        out=vsum[:], in0=vsum[:], scalar1=wn, scalar2=None, op0=OP.mult
                    )
                    for g in range(GL):
                        nc.vector.tensor_scalar(
                            out=aggT[e][:, g_lo[g] : g_hi[g]],
                            in0=aggT[e][:, g_lo[g] : g_hi[g]],
                            scalar1=vsum[:, g : g + 1], scalar2=bn,
                            op0=OP.mult, op1=OP.add,
                        )
                    nc.scalar.activation(
                        out=aggT[e][:, :NL], in_=aggT[e][:, :NL], func=AF.Relu
                    )
                    nc.vector.tensor_tensor(
                        out=h[e][:, :NL], in0=h[e][:, :NL], in1=aggT[e][:, :NL],
                        op=OP.add,
                    )

            # ---- node outputs (transpose h back to node-major) ----
            for ch in (range(NCH) if KSTAGE >= 1 else []):
                cw = cw_of(ch)
                nsb = wrk.tile([128, H2], F32, tag="nsb")
                for e in range(2):
                    pst = psp.tile([128, 128], F32, tag="ps")
                    nc.tensor.transpose(
                        out=pst[:cw, :],
                        in_=h[e][:, ch * 128 : ch * 128 + cw],
                        identity=eye[:, :],
                    )
                    nc.scalar.copy(out=nsb[:cw, e * H : (e + 1) * H], in_=pst[:cw, :])
                nc.sync.dma_start(
                    out=nodes_out[ch * 128 : ch * 128 + cw, :], in_=nsb[:cw, :]
                )

            # ---- pooling + heads ----
            if KSTAGE < 1:
                raise SystemExit(0)
            heads_sb = big.tile([GL, NHEAD], F32, tag="heads")
            emb = []  # per encoder: (gmean [128,GL], gmax [128,GL])
            for e in range(2):
                gmean = smp.tile([128, GL], F32, tag="st")
                gmax = smp.tile([128, GL], F32, tag="st")
                for g in range(GL):
                    nc.vector.reduce_sum(
                        out=gmean[:, g : g + 1], in_=h[e][:, g_lo[g] : g_hi[g]],
                        axis=mybir.AxisListType.X,
                    )
                    nc.vector.reduce_max(
                        out=gmax[:, g : g + 1], in_=h[e][:, g_lo[g] : g_hi[g]],
                        axis=mybir.AxisListType.X,
                    )
                nc.vector.tensor_tensor(
                    out=gmean[:], in0=gmean[:], in1=cntinv_sb[:], op=OP.mult
                )
                emb.append((gmean, gmax))

                # z1 = relu(W1.T z + b1)
                ps1 = psp.tile([128, GL], F32, tag="ps")
                nc.tensor.matmul(
                    out=ps1[:], lhsT=w1[:, (e * 2) * H : (e * 2) * H + H],
                    rhs=gmean[:], start=True, stop=False,
                )
                nc.tensor.matmul(
                    out=ps1[:], lhsT=w1[:, (e * 2 + 1) * H : (e * 2 + 1) * H + H],
                    rhs=gmax[:], start=False, stop=False,
                )
                nc.tensor.matmul(
                    out=ps1[:], lhsT=w1g[:, e * H : (e + 1) * H],
                    rhs=gfT[:], start=False, stop=True,
                )
                z1 = smp.tile([128, GL], F32, tag="z1")
                nc.scalar.activation(
                    out=z1[:], in_=ps1[:], func=AF.Relu, bias=b1[:, e : e + 1]
                )
                ps2 = psp.tile([64, GL], F32, tag="ps")
                nc.tensor.matmul(
                    out=ps2[:], lhsT=w2[:, e * 64 : (e + 1) * 64], rhs=z1[:],
                    start=True, stop=True,
                )
                z2 = smp.tile([64, GL], F32, tag="z1")
                nc.scalar.activation(
                    out=z2[:], in_=ps2[:], func=AF.Relu, bias=b2[:, e : e + 1]
                )
                na = NA if e == 0 else 2
                ps3 = psp.tile([NA, GL], F32, tag="ps")
                nc.tensor.matmul(
                    out=ps3[:na, :],
                    lhsT=w3[:, e * NA : e * NA + na] if e == 0 else w3[:, NA : NA + 2],
                    rhs=z2[:], start=True, stop=True,
                )
                lt = smp.tile([NA, GL], F32, tag="z1")
                nc.scalar.add(
                    out=lt[:na, :], in_=ps3[:na, :],
                    add=b3a[:, :] if e == 0 else b3c[:, :],
                )
                # transpose to graph-major [GL, na]
                pt = psp.tile([GL, NA], F32, tag="ps")
                nc.tensor.transpose(
                    out=pt[:, :na], in_=lt[:na, :], identity=eye[:na, :na]
                )
                if e == 0:
                    nc.scalar.copy(out=heads_sb[:, 0:NA], in_=pt[:, :NA])
                    # softmax over the NA axis (free dim)
                    mx = smp.tile([GL, 1], F32, tag="sm")
                    nc.vector.reduce_max(
                        out=mx[:], in_=pt[:, :NA], axis=mybir.AxisListType.X
                    )
                    ex = smp.tile([GL, NA], F32, tag="sm")
                    nc.vector.tensor_scalar(
                        out=ex[:], in0=pt[:, :NA], scalar1=mx[:], scalar2=None,
                        op0=OP.subtract,
                    )
                    nc.scalar.activation(out=ex[:], in_=ex[:], func=AF.Exp)
                    sm = smp.tile([GL, 1], F32, tag="sm")
                    nc.vector.reduce_sum(
                        out=sm[:], in_=ex[:], axis=mybir.AxisListType.X
                    )
                    nc.vector.reciprocal(out=sm[:], in_=sm[:])
                    nc.vector.tensor_scalar(
                        out=heads_sb[:, NA : 2 * NA], in0=ex[:], scalar1=sm[:],
                        scalar2=None, op0=OP.mult,
                    )
                else:
                    nc.scalar.copy(
                        out=heads_sb[:, 2 * NA : 2 * NA + 1], in_=pt[:, 0:1]
                    )
                    nc.scalar.activation(
                        out=heads_sb[:, 2 * NA + 1 : 2 * NA + 2], in_=pt[:, 1:2],
                        func=AF.Sigmoid,
                    )

            # emb transposes into heads block
            off = 2 * NA + 2
            for e in range(2):
                for k, tsr in enumerate(emb[e]):
                    pt = psp.tile([GL, 128], F32, tag="ps")
                    nc.tensor.transpose(out=pt[:], in_=tsr[:], identity=eye[:, :])
                    dst0 = off + e * H2 + k * H
                    nc.scalar.copy(
                        out=heads_sb[:, dst0 : dst0 + H], in_=pt[:, :]
                    )
            nc.sync.dma_start(out=heads_out[:], in_=heads_sb[:])

    nc.compile()
    return nc


# ----------------------------------------------------------------------------
# entry point
# ----------------------------------------------------------------------------

LAST = {}


def _run(x, edge_index, batch, global_features, params, cfg):
    in_maps, meta = preprocess(x, edge_index, batch, global_features, params, cfg)
    nc = build_kernel(meta)
    trace = os.environ.get("KTRACE") == "1"
    res = run_bass_kernel_spmd(
        nc, in_maps, core_ids=list(range(cfg["C"])), trace=trace
    )
    LAST["exec_time_ns"] = res.exec_time_ns
    LAST["profile_json"] = res.profile_json
    return _assemble(res.results, meta)


def _assemble(results, meta):
    GL = meta["GL"]
    nodes = np.concatenate([r["nodes_out"] for r in results], axis=0)
    heads = np.concatenate([r["heads_out"] for r in results], axis=0)
    a_nodes = np.ascontiguousarray(nodes[:, :H])
    c_nodes = np.ascontiguousarray(nodes[:, H:])
    action_logits = np.ascontiguousarray(heads[:, :NA])
    action_probs = np.ascontiguousarray(heads[:, NA : 2 * NA])
    state_value = np.ascontiguousarray(heads[:, 2 * NA])
    termination_prob = np.ascontiguousarray(heads[:, 2 * NA + 1])
    off = 2 * NA + 2
    a_emb = np.ascontiguousarray(heads[:, off : off + 2 * H])
    c_emb = np.ascontiguousarray(heads[:, off + 2 * H : off + 4 * H])
    return (
        action_logits, action_probs, state_value, termination_prob,
        a_emb, c_emb, a_nodes, c_nodes,
    )


def kernel(x, edge_index, batch, global_features, params):
    return _run(x, edge_index, batch, global_features, params, FULL_CFG)


# revision 14
# speedup vs baseline: 1.1775x; 1.1775x over previous
"""Trainium2 Bass kernel for the ActorCritic GNN (gnn_message_passing).

Strategy (8 NeuronCores, graph-data parallel):
  - Core c owns graphs [8c, 8c+8) == node rows [6250c, 6250(c+1)) (the batch
    vector's graph boundaries align with the 8-way node split).
  - Edges are partitioned by dst owner; all segment reductions (scatter-sum,
    GraphNorm, pooling) stay device-local.
  - Per conv layer: each core computes its local m' = (h @ W) * dinv rows,
    AllGathers the full [N, 2H] message table into DRAM, then processes its
    ~E/8 edges: bulk dma_gather of 1KB rows + one-hot scatter matmuls
    accumulating into PSUM windows of 128 dst nodes.
  - Both encoders (actor/critic) are fused: one gather serves both (rows hold
    [actor 128 | critic 128] features).
  - int16 gather indices -> the node table is processed as two halves
    (rows [0, 32768) and [32768, N)), edges bucketed by src half.
"""

import os
import numpy as np

import concourse.bacc as bacc
import concourse.bass as bass
import concourse.mybir as mybir
import concourse.tile as tile
from concourse import library_config
from concourse.bass_utils import run_bass_kernel_spmd

F32 = mybir.dt.float32
BF = mybir.dt.bfloat16
I16 = mybir.dt.int16
I32 = mybir.dt.int32
AF = mybir.ActivationFunctionType
OP = mybir.AluOpType

H = 128          # hidden width (fixed by the arch; layout assumes 128)
ND = 7
GF = 10
NA = 7
L = 3
EPS = 1e-5

FULL_CFG = dict(N=50000, E=800000, B=64, C=8, HALF=32768)


# ----------------------------------------------------------------------------
# host preprocessing
# ----------------------------------------------------------------------------

def _pack_gather_idx(e_flat):
    """Pack slot-ordered int16 indices (slot s = t*128 + p) into the
    [16, 8T] layout dma_gather reads: slot (p=16a+b, t) <- H[b, 8t+a]."""
    T = len(e_flat) // 128
    return e_flat.reshape(T, 8, 16).transpose(2, 0, 1).reshape(16, 8 * T)


def preprocess(x, edge_index, batch, global_features, params, cfg):
    N, E, B, C, HALFV = cfg["N"], cfg["E"], cfg["B"], cfg["C"], cfg["HALF"]
    NL = N // C
    GL = B // C
    NCH = (NL + 127) // 128          # windows / node chunks per core
    CWL = NL - 128 * (NCH - 1)       # width of last chunk

    src = np.asarray(edge_index[0], dtype=np.int64)
    dst = np.asarray(edge_index[1], dtype=np.int64)
    batch = np.asarray(batch, dtype=np.int64)
    x = np.asarray(x, dtype=np.float32)
    gfeat = np.asarray(global_features, dtype=np.float32)

    deg = np.bincount(dst, minlength=N).astype(np.float64) + 1.0
    dinv = (deg ** -0.5).astype(np.float32)

    # fold the GCN self-loop in as real edges: contribution dinv[d]*m'[d]
    allx = np.arange(N, dtype=np.int64)
    src = np.concatenate([src, allx])
    dst = np.concatenate([dst, allx])

    # graph boundaries; check the 8-way split is uniform and graph-aligned
    gb = np.searchsorted(batch, np.arange(B + 1))
    cs = np.array([gb[c * GL] for c in range(C)] + [N])
    assert np.all(np.diff(cs) == NL), f"non-uniform core node split {np.diff(cs)}"
    # local graph slices (must be identical across cores)
    gsl = gb[: C * GL].reshape(C, GL) - cs[:C, None]
    assert np.all(gsl == gsl[0]), "graph slices differ across cores"
    g_lo = gsl[0]
    g_hi = np.concatenate([gsl[0][1:], [NL]])
    g_cnt = (g_hi - g_lo).astype(np.float64)

    # --- edge bucketing: (core, window, src-half) ---
    core_e = dst // NL
    dstloc = dst - core_e * NL
    w_e = dstloc // 128
    off_e = (dstloc % 128).astype(np.float32)
    s_e = (src >= HALFV).astype(np.int64)

    key = (core_e * NCH + w_e) * 2 + s_e
    order = np.argsort(key, kind="stable")
    key_s = key[order]
    nbuckets = C * NCH * 2
    bstart = np.searchsorted(key_s, np.arange(nbuckets + 1))
    counts = np.diff(bstart).reshape(C, NCH, 2)

    T_star = np.ceil(counts / 128).astype(np.int64).max(axis=0)  # [NCH, 2]
    TT = int(T_star.sum())            # tiles per layer (static)
    IC = 8 * TT                       # idx columns

    src_s = src[order]
    off_s = off_e[order]

    idx_all = np.zeros((C, 16, IC), dtype=np.int16)
    dstoff = np.full((C, 128, TT), -1.0, dtype=np.float32)

    # tile index (tau) starts per (w, s); tiles are laid out per-half as one
    # stream (half 0 tiles of all windows, then half 1), so gather segments of
    # 8 tiles can span window boundaries and stay full.
    tiles_half = [int(T_star[:, s].sum()) for s in range(2)]
    tau0 = np.zeros((NCH, 2), dtype=np.int64)
    acc = [0, tiles_half[0]]
    for w in range(NCH):
        for s in range(2):
            tau0[w, s] = acc[s]
            acc[s] += T_star[w, s]

    for c in range(C):
        # build per-half padded streams in (w) order
        for s in range(2):
            parts_idx, parts_off = [], []
            for w in range(NCH):
                Tb = int(T_star[w, s])
                if Tb == 0:
                    continue
                b = (c * NCH + w) * 2 + s
                lo, hi = bstart[b], bstart[b + 1]
                n = hi - lo
                e_idx = np.zeros(128 * Tb, dtype=np.int16)
                e_off = np.full(128 * Tb, -1.0, dtype=np.float32)
                vals = src_s[lo:hi] - (HALFV if s == 1 else 0)
                e_idx[:n] = vals.astype(np.int16)
                e_off[:n] = off_s[lo:hi]
                parts_idx.append(e_idx)
                parts_off.append(e_off)
            if not parts_idx:
                continue
            stream_idx = np.concatenate(parts_idx)
            stream_off = np.concatenate(parts_off)
            ntile = len(stream_idx) // 128
            tbase = 0 if s == 0 else tiles_half[0]
            # idx cols: packed per 8-tile segment of this half's stream
            for t0 in range(0, ntile, 8):
                ts = min(8, ntile - t0)
                cc = 8 * (tbase + t0)
                idx_all[c, :, cc : cc + 8 * ts] = _pack_gather_idx(
                    stream_idx[128 * t0 : 128 * (t0 + ts)]
                )
            dstoff[c, :, tbase : tbase + ntile] = (
                stream_off.reshape(ntile, 128).T
            )

    idx_all_full = np.tile(idx_all, (1, 8, 1))  # [C, 128, IC]

    # --- per-core dense inputs ---
    xT = np.stack([x[cs[c] : cs[c + 1]].T for c in range(C)])           # [C,7,NL]
    gfT = np.stack([gfeat[c * GL : (c + 1) * GL].T for c in range(C)])  # [C,GF,GL]
    dinv_nd = np.zeros((C, 128, NCH), dtype=np.float32)
    for c in range(C):
        dv = dinv[cs[c] : cs[c + 1]]
        pad = np.zeros(NCH * 128, dtype=np.float32)
        pad[:NL] = dv
        dinv_nd[c] = pad.reshape(NCH, 128).T
    cntinv = np.broadcast_to(
        (1.0 / g_cnt).astype(np.float32)[None, :], (128, GL)
    ).copy()

    # --- parameters (replicated) ---
    def A(v):
        return np.ascontiguousarray(np.asarray(v, dtype=np.float32))

    pa, pc = params["actor_enc"], params["critic_enc"]
    inW = np.concatenate([A(pa["in_W"]), A(pc["in_W"])], axis=1)           # [7,2H]
    inb = np.stack([A(pa["in_b"]), A(pc["in_b"])], axis=1)                 # [H,2]
    convW = np.concatenate(
        [A(p["conv_W"][l]) for l in range(L) for p in (pa, pc)], axis=1
    )                                                                       # [H, L*2*H]
    convB = np.stack(
        [A(p["conv_b"][l]) for l in range(L) for p in (pa, pc)], axis=1
    )                                                                       # [H, L*2]
    normP = np.stack(
        [
            A(p[k][l])
            for l in range(L)
            for p in (pa, pc)
            for k in ("norm_a", "norm_w", "norm_b")
        ],
        axis=1,
    )                                                                       # [H, L*2*3]

    ha, hc = params["actor_head"], params["critic_head"]
    w1 = np.concatenate(
        [A(h_["W1"])[k * H : (k + 1) * H] for h_ in (ha, hc) for k in (0, 1)],
        axis=1,
    )                                                                       # [H, 4H]
    w1g = np.concatenate([A(ha["W1"])[2 * H :], A(hc["W1"])[2 * H :]], axis=1)  # [GF,2H]
    b1 = np.stack([A(ha["b1"]), A(hc["b1"])], axis=1)                      # [H,2]
    w2 = np.concatenate([A(ha["W2"]), A(hc["W2"])], axis=1)                # [H,128]
    b2 = np.stack([A(ha["b2"]), A(hc["b2"])], axis=1)                      # [64,2]
    w3 = np.concatenate([A(ha["W3"]), A(hc["W3"])], axis=1)                # [64,NA+2]
    b3a = A(ha["b3"])[:, None]                                             # [NA,1]
    b3c = A(hc["b3"])[:, None]                                             # [2,1]
    eye = np.eye(128, dtype=np.float32)

    in_maps = []
    for c in range(C):
        in_maps.append(
            {
                "xT": np.ascontiguousarray(xT[c]),
                "gfT": np.ascontiguousarray(gfT[c]),
                "idx_all": np.ascontiguousarray(idx_all_full[c]),
                "dstoff": np.ascontiguousarray(dstoff[c]),
                "dinv_nd": np.ascontiguousarray(dinv_nd[c]),
                "cntinv": cntinv,
                "inW": inW, "inb": inb, "convW": convW, "convB": convB,
                "normP": normP, "w1": w1, "w1g": w1g, "b1": b1,
                "w2": w2, "b2": b2, "w3": w3, "b3a": b3a, "b3c": b3c, "eye": eye,
            }
        )

    meta = dict(
        NL=NL, GL=GL, NCH=NCH, CWL=CWL, TT=TT, IC=IC,
        T_star=T_star, tiles_half=tiles_half, tau0=tau0,
        g_lo=g_lo.tolist(), g_hi=g_hi.tolist(),
        N=N, C=C, HALF=HALFV,
    )
    return in_maps, meta


# ----------------------------------------------------------------------------
# device kernel
# ----------------------------------------------------------------------------

def build_kernel(meta):
    NL, GL, NCH, CWL = meta["NL"], meta["GL"], meta["NCH"], meta["CWL"]
    TT, IC = meta["TT"], meta["IC"]
    T_star, tiles_half, tau0 = meta["T_star"], meta["tiles_half"], meta["tau0"]
    g_lo, g_hi = meta["g_lo"], meta["g_hi"]
    N, C, HALFV = meta["N"], meta["C"], meta["HALF"]
    H2 = 2 * H
    NHEAD = NA + NA + 1 + 1 + 2 * H2   # logits, probs, value, term, a_emb, c_emb

    nc = bacc.Bacc("TRN2", target_bir_lowering=False, debug=True)

    def P(name, shape, dtype=F32):
        return nc.declare_dram_parameter(name, shape, dtype, isOutput=False)

    xT_d = P("xT", [ND, NL])
    gfT_d = P("gfT", [GF, GL])
    idx_d = P("idx_all", [128, IC], I16)
    dstoff_d = P("dstoff", [128, TT])
    dinv_d = P("dinv_nd", [128, NCH])
    cntinv_d = P("cntinv", [128, GL])
    inW_d = P("inW", [ND, H2])
    inb_d = P("inb", [H, 2])
    convW_d = P("convW", [H, L * 2 * H])
    convB_d = P("convB", [H, L * 2])
    normP_d = P("normP", [H, L * 6])
    w1_d = P("w1", [H, 4 * H])
    w1g_d = P("w1g", [GF, H2])
    b1_d = P("b1", [H, 2])
    w2_d = P("w2", [H, 128])
    b2_d = P("b2", [64, 2])
    w3_d = P("w3", [64, NA + 2])
    b3a_d = P("b3a", [NA, 1])
    b3c_d = P("b3c", [2, 1])
    eye_d = P("eye", [128, 128])

    nodes_out = nc.declare_dram_parameter("nodes_out", [NL, H2], F32, isOutput=True)
    heads_out = nc.declare_dram_parameter("heads_out", [GL, NHEAD], F32, isOutput=True)

    shard = [nc.dram_tensor(f"shard{l}", [NL, H2], BF) for l in range(L)]
    mtab = [
        nc.dram_tensor(f"mtab{l}", [N, H2], BF, addr_space="Shared")
        for l in range(L)
    ]

    rg = [list(range(C))]

    with tile.TileContext(nc) as tc:
        nc.gpsimd.load_library(library_config.mlp)
        with (
            tc.tile_pool(name="const", bufs=1) as cst,
            tc.tile_pool(name="big", bufs=1) as big,
            tc.tile_pool(name="msg", bufs=4) as msgp,
            tc.tile_pool(name="oh", bufs=8) as ohp,
            tc.tile_pool(name="wrk", bufs=3) as wrk,
            tc.tile_pool(name="small", bufs=8) as smp,
            tc.tile_pool(name="psum", bufs=6, space="PSUM") as psp,
        ):
            # ---- constants to SBUF ----
            def load(dram, shape, dtype=F32):
                nm = f"c_{dram.name}"
                t = cst.tile(shape, dtype, tag=nm, name=nm)
                nc.sync.dma_start(out=t[:], in_=dram[:])
                return t

            xT = load(xT_d, [ND, NL])
            gfT = load(gfT_d, [GF, GL])
            idx_sb = load(idx_d, [128, IC], I16)
            dstoff_sb = load(dstoff_d, [128, TT])
            dinv_sb = load(dinv_d, [128, NCH])
            cntinv_sb = load(cntinv_d, [128, GL])
            inW = load(inW_d, [ND, H2])
            inb = load(inb_d, [H, 2])
            convW = load(convW_d, [H, L * 2 * H])
            convB = load(convB_d, [H, L * 2])
            normP = load(normP_d, [H, L * 6])
            w1 = load(w1_d, [H, 4 * H])
            w1g = load(w1g_d, [GF, H2])
            b1 = load(b1_d, [H, 2])
            w2 = load(w2_d, [H, 128])
            b2 = load(b2_d, [64, 2])
            w3 = load(w3_d, [64, NA + 2])
            b3a = load(b3a_d, [NA, 1])
            b3c = load(b3c_d, [2, 1])
            eye = load(eye_d, [128, 128])

            iota_i = cst.tile([128, 128], I32)
            nc.gpsimd.iota(iota_i[:], pattern=[[1, 128]], base=0, channel_multiplier=0)
            iota_b = cst.tile([128, 128], BF)
            nc.vector.tensor_copy(out=iota_b[:], in_=iota_i[:])
            eps_t = cst.tile([128, 1], F32)
            nc.vector.memset(eps_t[:], EPS)

            h = [big.tile([128, NL], F32, tag=f"h{e}", name=f"h{e}") for e in range(2)]
            aggT = [big.tile([128, NL], F32, tag=f"agg{e}", name=f"agg{e}") for e in range(2)]

            def cw_of(ch):
                return CWL if ch == NCH - 1 else 128

            # ---- input layer: h0 = relu(x @ in_W + in_b), feature-major ----
            NJ = (NL + 511) // 512
            for e in range(2):
                for j in range(NJ):
                    j0 = j * 512
                    wj = min(512, NL - j0)
                    ps = psp.tile([128, 512], F32, tag="ps")
                    nc.tensor.matmul(
                        out=ps[:, :wj],
                        lhsT=inW[:, e * H : (e + 1) * H],
                        rhs=xT[:, j0 : j0 + wj],
                        start=True, stop=True,
                    )
                    nc.scalar.activation(
                        out=h[e][:, j0 : j0 + wj], in_=ps[:, :wj],
                        func=AF.Relu, bias=inb[:, e : e + 1], scale=1.0,
                    )

            # ---- conv layers ----
            KSTAGE = int(os.environ.get("KSTAGE", "99"))
            NLAYERS = 0 if KSTAGE < 2 else (1 if KSTAGE < 6 else L)
            for l in range(NLAYERS):
                # stage B: m' = (h @ W_l) * dinv -> shard -> AllGather
                for ch in range(NCH):
                    cw = cw_of(ch)
                    msb = wrk.tile([128, H2], BF, tag="msb")
                    for e in range(2):
                        psb = psp.tile([128, 128], F32, tag="ps")
                        nc.tensor.matmul(
                            out=psb[:cw, :],
                            lhsT=h[e][:, ch * 128 : ch * 128 + cw],
                            rhs=convW[:, (l * 2 + e) * H : (l * 2 + e + 1) * H],
                            start=True, stop=True,
                        )
                        nc.scalar.activation(
                            out=msb[:cw, e * H : (e + 1) * H], in_=psb[:cw, :],
                            func=AF.Copy, scale=dinv_sb[:cw, ch : ch + 1],
                        )
                    nc.sync.dma_start(
                        out=shard[l][ch * 128 : ch * 128 + cw, :], in_=msb[:cw, :]
                    )
                if KSTAGE >= 3:
                    nc.gpsimd.collective_compute(
                        "AllGather", OP.bypass, replica_groups=rg,
                        ins=[shard[l][:]], outs=[mtab[l][:]],
                    )

                if KSTAGE < 4:
                    continue

                # stage D: edge pass. Gather segments are 8 tiles regardless
                # of window boundaries (one dma_gather per segment, lazily
                # emitted before the first matmul that needs it).
                seg_tiles = {}   # (s, k) -> msg tile

                def get_seg(s, k):
                    if (s, k) in seg_tiles:
                        return seg_tiles[(s, k)]
                    nt = tiles_half[s]
                    ts = min(8, nt - k * 8)
                    msg = msgp.tile([128, 8, H2], BF, tag="msg", name="msg")
                    base = mtab[l][0:HALFV, :] if s == 0 else mtab[l][HALFV:N, :]
                    tb = (0 if s == 0 else tiles_half[0]) + k * 8
                    nc.gpsimd.dma_gather(
                        msg[:, :ts, :], base,
                        idx_sb[:, 8 * tb : 8 * (tb + ts)],
                        128 * ts, 128 * ts, H2,
                    )
                    seg_tiles[(s, k)] = msg
                    return msg

                half_tile_pos = [0, 0]  # running tile index within each half
                for w in range(NCH):
                    cw = cw_of(w)
                    psw = psp.tile([128, H2], F32, tag="ps")
                    n_tiles = int(T_star[w, 0] + T_star[w, 1])
                    done = 0
                    for s in range(2):
                        Tb = int(T_star[w, s])
                        t0_ = int(tau0[w, s])
                        for t in range(Tb):
                            pos = half_tile_pos[s]
                            msg = get_seg(s, pos // 8)
                            oh = ohp.tile([128, 128], BF, tag="oh")
                            nc.vector.tensor_scalar(
                                out=oh[:], in0=iota_b[:],
                                scalar1=dstoff_sb[:, t0_ + t : t0_ + t + 1],
                                scalar2=None, op0=OP.is_equal,
                            )
                            nc.tensor.matmul(
                                out=psw[:, :], lhsT=oh[:],
                                rhs=msg[:, pos % 8, :],
                                start=(done == 0), stop=(done == n_tiles - 1),
                            )
                            done += 1
                            half_tile_pos[s] += 1
                    # epilogue: agg = scatter * dinv  (self-loop folded in as
                    # edges; conv bias folded into the GraphNorm stats)
                    aw = wrk.tile([128, H2], F32, tag="aw")
                    nc.scalar.activation(
                        out=aw[:cw, :], in_=psw[:cw, :], func=AF.Copy,
                        scale=dinv_sb[:cw, w : w + 1],
                    )
                    for e in range(2):
                        pst = psp.tile([128, 128], F32, tag="ps")
                        nc.tensor.transpose(
                            out=pst[:, :cw],
                            in_=aw[:cw, e * H : (e + 1) * H],
                            identity=eye[:cw, :cw],
                        )
                        nc.scalar.copy(
                            out=aggT[e][:, w * 128 : w * 128 + cw], in_=pst[:, :cw]
                        )

                # stage E: GraphNorm + relu + skip (feature-major, in place)
                if KSTAGE < 5:
                    continue
                for e in range(2):
                    al = normP[:, (l * 2 + e) * 3 + 0 : (l * 2 + e) * 3 + 1]
                    wn = normP[:, (l * 2 + e) * 3 + 1 : (l * 2 + e) * 3 + 2]
                    bn = normP[:, (l * 2 + e) * 3 + 2 : (l * 2 + e) * 3 + 3]
                    bc = convB[:, l * 2 + e : l * 2 + e + 1]

                    gsum = smp.tile([128, GL], F32, tag="st")
                    for g in range(GL):
                        nc.vector.reduce_sum(
                            out=gsum[:, g : g + 1],
                            in_=aggT[e][:, g_lo[g] : g_hi[g]],
                            axis=mybir.AxisListType.X,
                        )
                    sg = smp.tile([128, GL], F32, tag="st")
                    nc.vector.tensor_tensor(
                        out=sg[:], in0=gsum[:], in1=cntinv_sb[:], op=OP.mult
                    )
                    # sg = alpha*(mean + b_conv) - b_conv
                    nc.vector.tensor_scalar(
                        out=sg[:], in0=sg[:], scalar1=bc, scalar2=al,
                        op0=OP.add, op1=OP.mult,
                    )
                    nc.vector.tensor_scalar(
                        out=sg[:], in0=sg[:], scalar1=bc, scalar2=None,
                        op0=OP.subtract,
                    )
                    for g in range(GL):
                        nc.vector.tensor_scalar(
                            out=aggT[e][:, g_lo[g] : g_hi[g]],
                            in0=aggT[e][:, g_lo[g] : g_hi[g]],
                            scalar1=sg[:, g : g + 1], scalar2=None,
                            op0=OP.subtract,
                        )
                    vsum = smp.tile([128, GL], F32, tag="st")
                    sq = wrk.tile([128, 1024], F32, tag="sq")
                    for g in range(GL):
                        nc.scalar.activation(
                            out=sq[:, : g_hi[g] - g_lo[g]],
                            in_=aggT[e][:, g_lo[g] : g_hi[g]],
                            func=AF.Square,
                            accum_out=vsum[:, g : g + 1],
                        )
                    nc.vector.tensor_tensor(
                        out=vsum[:], in0=vsum[:], in1=cntinv_sb[:], op=OP.mult
                    )
                    nc.scalar.activation(out=vsum[:], in_=vsum[:], func=AF.Sqrt, bias=eps_t[:])
                    nc.vector.reciprocal(out=vsum[:], in_=vsum[:])
                    nc.vector.tensor_scalar(
                        out=vsum[:], in0=vsum[:], scalar1=wn, scalar2=None, op0=OP.mult
                    )
                    for g in range(GL):
                        nc.vector.tensor_scalar(
                            out=aggT[e][:, g_lo[g] : g_hi[g]],
                            in0=aggT[e][:, g_lo[g] : g_hi[g]],
                            scalar1=vsum[:, g : g + 1], scalar2=bn,
                            op0=OP.mult, op1=OP.add,
                        )
                    nc.scalar.activation(
                        out=aggT[e][:, :NL], in_=aggT[e][:, :NL], func=AF.Relu
                    )
                    nc.vector.tensor_tensor(
                        out=h[e][:, :NL], in0=h[e][:, :NL], in1=aggT[e][:, :NL],
                        op=OP.add,
                    )

            # ---- node outputs (transpose h back to node-major) ----
            for ch in (range(NCH) if KSTAGE >= 1 else []):
                cw = cw_of(ch)
                nsb = wrk.tile([128, H2], F32, tag="nsb")
                for e in range(2):
                    pst = psp.tile([128, 128], F32, tag="ps")
                    nc.tensor.transpose(
                        out=pst[:cw, :],
                        in_=h[e][:, ch * 128 : ch * 128 + cw],
                        identity=eye[:, :],
                    )
                    nc.scalar.copy(out=nsb[:cw, e * H : (e + 1) * H], in_=pst[:cw, :])
                nc.sync.dma_start(
                    out=nodes_out[ch * 128 : ch * 128 + cw, :], in_=nsb[:cw, :]
                )

            # ---- pooling + heads ----
            if KSTAGE < 1:
                raise SystemExit(0)
            heads_sb = big.tile([GL, NHEAD], F32, tag="heads")
            emb = []  # per encoder: (gmean [128,GL], gmax [128,GL])
            for e in range(2):
                gmean = smp.tile([128, GL], F32, tag="st")
                gmax = smp.tile([128, GL], F32, tag="st")
                for g in range(GL):
                    nc.vector.reduce_sum(
                        out=gmean[:, g : g + 1], in_=h[e][:, g_lo[g] : g_hi[g]],
                        axis=mybir.AxisListType.X,
                    )
                    nc.vector.reduce_max(
                        out=gmax[:, g : g + 1], in_=h[e][:, g_lo[g] : g_hi[g]],
                        axis=mybir.AxisListType.X,
                    )
                nc.vector.tensor_tensor(
                    out=gmean[:], in0=gmean[:], in1=cntinv_sb[:], op=OP.mult
                )
                emb.append((gmean, gmax))

                # z1 = relu(W1.T z + b1)
                ps1 = psp.tile([128, GL], F32, tag="ps")
                nc.tensor.matmul(
                    out=ps1[:], lhsT=w1[:, (e * 2) * H : (e * 2) * H + H],
                    rhs=gmean[:], start=True, stop=False,
                )
                nc.tensor.matmul(
                    out=ps1[:], lhsT=w1[:, (e * 2 + 1) * H : (e * 2 + 1) * H + H],
                    rhs=gmax[:], start=False, stop=False,
                )
                nc.tensor.matmul(
                    out=ps1[:], lhsT=w1g[:, e * H : (e + 1) * H],
                    rhs=gfT[:], start=False, stop=True,
                )
                z1 = smp.tile([128, GL], F32, tag="z1")
                nc.scalar.activation(
                    out=z1[:], in_=ps1[:], func=AF.Relu, bias=b1[:, e : e + 1]
                )
                ps2 = psp.tile([64, GL], F32, tag="ps")
                nc.tensor.matmul(
                    out=ps2[:], lhsT=w2[:, e * 64 : (e + 1) * 64], rhs=z1[:],
                    start=True, stop=True,
                )
                z2 = smp.tile([64, GL], F32, tag="z1")
                nc.scalar.activation(
                    out=z2[:], in_=ps2[:], func=AF.Relu, bias=b2[:, e : e + 1]
                )
                na = NA if e == 0 else 2
                ps3 = psp.tile([NA, GL], F32, tag="ps")
                nc.tensor.matmul(
                    out=ps3[:na, :],
                    lhsT=w3[:, e * NA : e * NA + na] if e == 0 else w3[:, NA : NA + 2],
                    rhs=z2[:], start=True, stop=True,
                )
                lt = smp.tile([NA, GL], F32, tag="z1")
                nc.scalar.add(
                    out=lt[:na, :], in_=ps3[:na, :],
                    add=b3a[:, :] if e == 0 else b3c[:, :],
                )
                # transpose to graph-major [GL, na]
                pt = psp.tile([GL, NA], F32, tag="ps")
                nc.tensor.transpose(
                    out=pt[:, :na], in_=lt[:na, :], identity=eye[:na, :na]
                )
                if e == 0:
                    nc.scalar.copy(out=heads_sb[:, 0:NA], in_=pt[:, :NA])
                    # softmax over the NA axis (free dim)
                    mx = smp.tile([GL, 1], F32, tag="sm")
                    nc.vector.reduce_max(
                        out=mx[:], in_=pt[:, :NA], axis=mybir.AxisListType.X
                    )
                    ex = smp.tile([GL, NA], F32, tag="sm")
                    nc.vector.tensor_scalar(
                        out=ex[:], in0=pt[:, :NA], scalar1=mx[:], scalar2=None,
                        op0=OP.subtract,
                    )
                    nc.scalar.activation(out=ex[:], in_=ex[:], func=AF.Exp)
                    sm = smp.tile([GL, 1], F32, tag="sm")
                    nc.vector.reduce_sum(
                        out=sm[:], in_=ex[:], axis=mybir.AxisListType.X
                    )
                    nc.vector.reciprocal(out=sm[:], in_=sm[:])
                    nc.vector.tensor_scalar(
                        out=heads_sb[:, NA : 2 * NA], in0=ex[:], scalar1=sm[:],
                        scalar2=None, op0=OP.mult,
                    )
                else:
                    nc.scalar.copy(
                        out=heads_sb[:, 2 * NA : 2 * NA + 1], in_=pt[:, 0:1]
                    )
                    nc.scalar.activation(
                        out=heads_sb[:, 2 * NA + 1 : 2 * NA + 2], in_=pt[:, 1:2],
                        func=AF.Sigmoid,
                    )

            # emb transposes into heads block
            off = 2 * NA + 2
            for e in range(2):
                for k, tsr in enumerate(emb[e]):
                    pt = psp.tile([GL, 128], F32, tag="ps")
                    nc.tensor.transpose(out=pt[:], in_=tsr[:], identity=eye[:, :])
                    dst0 = off + e * H2 + k * H
                    nc.scalar.copy(
                        out=heads_sb[:, dst0 : dst0 + H], in_=pt[:, :]
                    )
            nc.sync.dma_start(out=heads_out[:], in_=heads_sb[:])

    nc.compile()
    return nc


# ----------------------------------------------------------------------------
# entry point
# ----------------------------------------------------------------------------

LAST = {}


def _run(x, edge_index, batch, global_features, params, cfg):
    in_maps, meta = preprocess(x, edge_index, batch, global_features, params, cfg)
    nc = build_kernel(meta)
    trace = os.environ.get("KTRACE") == "1"
    res = run_bass_kernel_spmd(
        nc, in_maps, core_ids=list(range(cfg["C"])), trace=trace
    )
    LAST["exec_time_ns"] = res.exec_time_ns
    LAST["profile_json"] = res.profile_json
    return _assemble(res.results, meta)


def _assemble(results, meta):
    GL = meta["GL"]
    nodes = np.concatenate([r["nodes_out"] for r in results], axis=0)
    heads = np.concatenate([r["heads_out"] for r in results], axis=0)
    a_nodes = np.ascontiguousarray(nodes[:, :H])
    c_nodes = np.ascontiguousarray(nodes[:, H:])
    action_logits = np.ascontiguousarray(heads[:, :NA])
    action_probs = np.ascontiguousarray(heads[:, NA : 2 * NA])
    state_value = np.ascontiguousarray(heads[:, 2 * NA])
    termination_prob = np.ascontiguousarray(heads[:, 2 * NA + 1])
    off = 2 * NA + 2
    a_emb = np.ascontiguousarray(heads[:, off : off + 2 * H])
    c_emb = np.ascontiguousarray(heads[:, off + 2 * H : off + 4 * H])
    return (
        action_logits, action_probs, state_value, termination_prob,
        a_emb, c_emb, a_nodes, c_nodes,
    )


def kernel(x, edge_index, batch, global_features, params):
    return _run(x, edge_index, batch, global_features, params, FULL_CFG)
